# revision 53
# baseline (speedup 1.0000x reference)
"""Fused PVT-style transformer block kernel for Trainium2 (8 NeuronCores).

Sharding: pure data-parallel over batch B=8 -> one batch item per core.
Layout: channel-major ("transposed") activations [C(part), N(free)] throughout;
host pre-transposes x and relative_pos, post-transposes the output.

Per-core pipeline (N=3136=56x56 tokens, C=256, 4 heads x 64, KV=784=28x28,
HID=1024):
  LN1 (PE ones-matmul stats + PE K=1 broadcast + DVE apply; gamma/beta folded
  into downstream weights) -> q/k/v projections (bf16 PE) with the 2x2/s2
  spatial-reduction depthwise conv on DVE -> flash attention per (head,
  q-tile): scores^T = k^T.T @ q^T with rel-pos bias added via identity matmul
  into PSUM, exp on ACT (no max-subtraction: logits are O(1)), AV matmul with
  ones-row-augmented V giving the softmax denominator for free -> wo
  projection + residual -> LN2 -> conv1x1 (+gelu+bn1) -> 3x3 depthwise conv
  split across PE (fp32r diagonal matmuls into PSUM) and DVE (fused
  scalar_tensor_tensor taps) -> gelu -> conv1x1 (bn2/pbn folded) + residual ->
  final 3x3 depthwise conv (residual folded into center tap) -> output.
"""

import numpy as np
import ml_dtypes

B, N, C, NH, DH, KV, HID = 8, 3136, 256, 4, 64, 784, 1024
HS = WS = 56
NT = 448            # n-tile (8 rows of 56)
NNT = N // NT       # 7
KT = 112            # kv tile
NKT = KV // KT      # 7
EPS = 1e-5
BF16 = ml_dtypes.bfloat16

DW_PE_TILES = (3, 4, 5, 6, 7)   # HID ch-tiles whose dwconv runs on PE
DW_POOL_TILES = (1,)            # dw tiles on Pool via mul+add TT pairs
BLK_SLOW_TILES = ()             # blk dwconv ch-tiles on DVE (rest: PE)

TAPS = [(dy, dx) for dy in (-1, 0, 1) for dx in (-1, 0, 1) if (dy, dx) != (0, 0)]


def tap_idx(dy, dx):
    return (dy + 1) * 3 + (dx + 1)


def _build_program(iters=1, feedback=False):
    """feedback=True: body i>0 reads its x from fT (previous body's output)
    instead of xT, forcing a true serial dependency chain across bodies —
    used only for timing (defeats any cross-body dead-code elimination)."""
    import concourse.bacc as bacc
    import concourse.mybir as mybir
    import concourse.tile as tile
    from contextlib import ExitStack

    dt = mybir.dt
    F32, BF, F32R = dt.float32, dt.bfloat16, dt.float32r
    Alu = mybir.AluOpType
    Act = mybir.ActivationFunctionType
    DR = mybir.MatmulPerfMode.DoubleRow
    WSC = 32.0   # host-side fp8 weight pre-scale, undone at evict

    nc = bacc.Bacc("TRN2", target_bir_lowering=False, debug=False, num_devices=8)

    def din(name, shape, dtype):
        return nc.dram_tensor(name, shape, dtype, kind="ExternalInput")

    F8 = dt.float8e4

    xT_d = din("xT", [C, N], F32)
    rpT_d = din("rpT", [NH, NNT, KT, NKT, NT], BF)
    # fp8 projection weights, pre-scaled x32 host-side (de-scaled at evict)
    # and laid out j-major for DoubleRow: [128, j=2, out] with j the
    # contraction half (channels 128j..128j+127).
    wq8_d = din("wq8", [128, 2 * C], F8)
    wk8_d = din("wk8", [128, 2 * C], F8)
    wv8_d = din("wv8", [128, 2 * C], F8)
    woT_d = din("woT", [C, C], BF)
    w18_d = din("w18", [128, 2 * HID], F8)
    w28_d = din("w28", [128, 8 * C], F8)
    b2rr_d = din("b2rr", [1, C], BF)
    bq_d = din("bq", [C], F32)
    bk_d = din("bk", [C], F32)
    bo_d = din("bo", [C], F32)
    b1_d = din("b1", [HID], F32)
    ncol_d = din("ncol", [HID], F32)
    srw_d = din("srw", [C, 4], F32)
    dw9_d = din("dw9", [HID, 9], F32)
    dwb_d = din("dwb", [HID], F32)
    bk9_d = din("bk9", [C, 9], F32)
    bkb_d = din("bkb", [1, C], BF)
    bkbc_d = din("bkbc", [C], F32)
    eyeb_d = din("eyeb", [128, 128], BF)
    onesr_d = din("onesr", [1, 128], BF)
    sselw_d = din("sselw", [NNT, NNT * 128], BF)
    fT_d = nc.dram_tensor("fT", [C, N], F32, kind="ExternalOutput")

    def r32(ap):
        return ap.bitcast(F32R)

    with tile.TileContext(nc) as tc, ExitStack() as octx:
        wpool = octx.enter_context(tc.tile_pool(name="weights", bufs=1))
        persist = octx.enter_context(tc.tile_pool(name="persist", bufs=1))
        digp = octx.enter_context(tc.tile_pool(name="diag", bufs=1))

        # input tiles first: LN1's first stats matmul needs xres[*][:, :448];
        # emitting these DMAs before the ~1.7MB of weight loads removes the
        # 29us startup stall (weights aren't needed until q/k/v projections).
        xres = [persist.tile([128, N], F32, tag=f"xres{t}", name=f"xres{t}") for t in range(2)]
        for nt in range(NNT):
            cs = slice(nt * NT, (nt + 1) * NT)
            for t in range(2):
                nc.sync.dma_start(out=xres[t][:, cs], in_=xT_d[t * 128:(t + 1) * 128, cs])

        def wload(dram_ap, shape, dtype, tag):
            t = wpool.tile(shape, dtype, tag=tag, name=tag)
            nc.sync.dma_start(out=t[:], in_=dram_ap)
            return t

        # LN1's broadcast matmuls need ssel before any other weight arrives
        ssel_all = wload(sselw_d[:, :], [NNT, NNT * 128], BF, "sselw")
        ssel = [ssel_all[:, j * 128:(j + 1) * 128] for j in range(NNT)]
        wq3 = wload(wq8_d[:, :], [128, 2 * C], F8, "wq8")[:].rearrange(
            "p (j m) -> p j m", j=2)
        wk3 = wload(wk8_d[:, :], [128, 2 * C], F8, "wk8")[:].rearrange(
            "p (j m) -> p j m", j=2)
        wv3 = wload(wv8_d[:, :], [128, 2 * C], F8, "wv8")[:].rearrange(
            "p (j m) -> p j m", j=2)
        wo_sb = [wload(woT_d[k * 128:(k + 1) * 128, :], [128, C], BF, f"wo{k}") for k in range(2)]
        w13 = wload(w18_d[:, :], [128, 2 * HID], F8, "w18")[:].rearrange(
            "p (j m) -> p j m", j=2)
        w23 = wload(w28_d[:, :], [128, 8 * C], F8, "w28")[:].rearrange(
            "p (q j m) -> p q j m", q=4, j=2)
        b2rr = wload(b2rr_d[:, :], [1, C], BF, "b2rr")
        eyeb = wload(eyeb_d[:, :], [128, 128], BF, "eyeb")
        onesr = wload(onesr_d[:, :], [1, 128], BF, "onesr")
        srw_sb = [wload(srw_d.ap().rearrange("(t p) k -> p t k", p=128)[:, t, :],
                        [128, 4], F32, f"srw{t}") for t in range(2)]
        dw9_sb = [wload(dw9_d.ap().rearrange("(t p) k -> p t k", p=128)[:, t, :],
                        [128, 9], F32, f"dw9_{t}") for t in range(8)]
        bk9_sb = [wload(bk9_d.ap().rearrange("(t p) k -> p t k", p=128)[:, t, :],
                        [128, 9], F32, f"bk9_{t}") for t in range(2)]

        def vload(dram, n, tag):
            t = wpool.tile([128, n // 128], F32, tag=tag, name=tag)
            nc.sync.dma_start(out=t[:], in_=dram.ap().rearrange("(t p) -> p t", p=128))
            return t

        bq_sb = vload(bq_d, C, "bq")
        bk_sb = vload(bk_d, C, "bk")
        bo_sb = vload(bo_d, C, "bo")
        b1_sb = vload(b1_d, HID, "b1")
        ncol_sb = vload(ncol_d, HID, "ncol")
        dwb_sb = vload(dwb_d, HID, "dwb")
        bkb_row = wload(bkb_d[:, :], [1, C], BF, "bkb")
        bkb_col = vload(bkbc_d, C, "bkbc")
        onesn = wpool.tile([1, NT], BF, tag="onesn")
        nc.vector.memset(onesn[:], 1.0)
        # ones row living at partition 64: matmul lhsT/rhs must share their
        # base partition, and the softmax denominator row sits at partition 64
        onesr64 = wpool.tile([65, 128], BF, tag="onesr64")
        nc.vector.memset(onesr64[64:65, :], 1.0)
        onescol = wpool.tile([128, 1], F32, tag="onescol")
        nc.vector.memset(onescol[:], 1.0)
        onescol_b = wpool.tile([128, 1], BF, tag="onescol_b")
        nc.vector.memset(onescol_b[:], 1.0)
        # ones-selector windows: zsel[:, 6-nt:13-nt] is [128, NNT] with ones in
        # column nt -> stats matmul writes partition nt of a [NNT, NT] PSUM
        # tile (matmul out base partition must be 0/32/64, so row-packing goes
        # through the stationary operand instead).
        zsel_f = wpool.tile([128, 2 * NNT - 1], F32, tag="zsel_f")
        nc.vector.memset(zsel_f[:], 0.0)
        nc.vector.memset(zsel_f[:, NNT - 1:NNT], 1.0)
        zsel_b = wpool.tile([128, 2 * NNT - 1], BF, tag="zsel_b")
        nc.vector.memset(zsel_b[:], 0.0)
        nc.vector.memset(zsel_b[:, NNT - 1:NNT], 1.0)
        onespad = wpool.tile([128, HS], BF, tag="onespad")
        nc.vector.memset(onespad[:], 1.0)
        epscol = wpool.tile([128, 1], F32, tag="epscol")
        nc.vector.memset(epscol[:], EPS)

        # diag matrices for the PE dwconv paths; built during the qkv
        # window (ACT/DVE idle there) so the MLP never stalls on them
        diag_all = {}

        def build_diag(key, w9_sb):
            diag = []
            for t in range(9):
                dg = digp.tile([128, 128], BF, tag=f"dg{key}_{t}",
                               name=f"dg{key}_{t}")
                if t % 2 == 0:
                    nc.vector.tensor_scalar_mul(dg[:], eyeb[:], w9_sb[:, t:t + 1])
                else:
                    nc.scalar.activation(dg[:], eyeb[:], Act.Identity,
                                         bias=0.0, scale=w9_sb[:, t:t + 1])
                diag.append(dg)
            diag_all[key] = diag

        # LN output (fp8, un-affine'd), j-major [p, ct, n] for DoubleRow
        # moving operands; reused for LN1 and LN2
        h8 = persist.tile([128, 2 * N], F8, tag="h8", name="h8")
        h3 = h8[:].rearrange("p (j n) -> p j n", j=2)
        # bf16 staging copies of x so LN stats matmuls run at bf16 rate
        # (the BIR verifier rejects f32r fed straight from a DMA)
        xstage = [persist.tile([128, N], BF, tag=f"xstg{t}", name=f"xstg{t}")
                  for t in range(2)]


        def body(suffix):
            if not suffix.endswith("i0"):
                src_d = fT_d if feedback else xT_d
                for t in range(2):
                    for nt in range(NNT):
                        cs = slice(nt * NT, (nt + 1) * NT)
                        nc.sync.dma_start(out=xres[t][:, cs],
                                          in_=src_d[t * 128:(t + 1) * 128, cs])
            run_stages(suffix)

        def layer_norm(suffix, sq_src=None, st0_src=None):
            """hbuf <- (xres - mean_c) * rsqrt(var_c + eps).

            Stats land in partition-packed [NNT, NT] PSUM tiles (one partition
            per n-tile), so the per-row pipeline (square/var/sqrt/recip) runs
            once over [NNT, NT] instead of NNT times over [1, NT]. Stats
            matmuls use f32r (1 cyc/row at >=256 moving vs 4 for fp32);
            squares run on the otherwise-idle Pool engine."""
            with ExitStack() as ctx:
                sqp = ctx.enter_context(tc.tile_pool(name=f"ln_sq{suffix}", bufs=2))
                stp = ctx.enter_context(tc.tile_pool(name=f"ln_st{suffix}", bufs=1, space="PSUM"))
                bcp = ctx.enter_context(tc.tile_pool(name=f"ln_bc{suffix}", bufs=2, space="PSUM"))
                rowp = ctx.enter_context(tc.tile_pool(name=f"ln_row{suffix}", bufs=1))
                tmpp = ctx.enter_context(tc.tile_pool(name=f"ln_tmp{suffix}", bufs=3))
                st0 = st0_src if st0_src is not None else stp.tile(
                    [NNT, NT], F32, tag="st0", name="st0")
                st1 = stp.tile([NNT, NT], F32, tag="st1", name="st1")
                for nt in range(NNT):
                    cs = slice(nt * NT, (nt + 1) * NT)
                    for ct in range(2):
                        if st0_src is not None:
                            continue
                        ceng = nc.gpsimd if (2 * nt + ct) % 2 == 0 else nc.vector
                        ceng.tensor_copy(xstage[ct][:, cs], xres[ct][:, cs])
                        nc.tensor.matmul(st0[:], zsel_b[:, NNT - 1 - nt:2 * NNT - 1 - nt],
                                         xstage[ct][:, cs],
                                         start=(nt == 0 and ct == 0),
                                         stop=(nt == NNT - 1 and ct == 1))
                    for ct in range(2):
                        if sq_src is None:
                            sq = sqp.tile([128, NT], BF)
                            k3 = (2 * nt + ct) % 3
                            if k3 == 0:
                                nc.scalar.square(sq[:], xstage[ct][:, cs])
                            else:
                                eng = nc.vector if k3 == 1 else nc.gpsimd
                                eng.tensor_mul(sq[:], xstage[ct][:, cs],
                                               xstage[ct][:, cs])
                            sqv = sq[:]
                        else:
                            sqv = sq_src[ct][:, cs]
                        nc.tensor.matmul(st1[:], zsel_b[:, NNT - 1 - nt:2 * NNT - 1 - nt],
                                         sqv,
                                         start=(nt == 0 and ct == 0),
                                         stop=(nt == NNT - 1 and ct == 1))
                m2 = rowp.tile([NNT, NT], F32, tag="m2", name="m2")
                nc.scalar.activation(m2[:], st0[:], Act.Square, scale=1.0 / C)
                var = rowp.tile([NNT, NT], F32, tag="var", name="var")
                nc.vector.scalar_tensor_tensor(var[:], st1[:], 1.0 / C, m2[:],
                                               op0=Alu.mult, op1=Alu.subtract)
                # rstd = exp(-0.5*ln(var+eps)): Ln and Exp share one ACT
                # table set with the attention exp, so no Sqrt table load
                lg = rowp.tile([NNT, NT], F32, tag="lg", name="lg")
                nc.scalar.activation(lg[:], var[:], Act.Ln, bias=epscol[0:NNT, :])
                arow = rowp.tile([NNT, NT], BF, tag="arow", name="arow")
                with nc.allow_low_precision("bf16 rstd broadcast row"):
                    nc.scalar.activation(arow[:], lg[:], Act.Exp, scale=-0.5)
                crow = rowp.tile([NNT, NT], BF, tag="crow", name="crow")
                nc.vector.scalar_tensor_tensor(crow[:], st0[:], -1.0 / C, arow[:],
                                               op0=Alu.mult, op1=Alu.mult)
                for nt in range(NNT):
                    cs = slice(nt * NT, (nt + 1) * NT)
                    pc = bcp.tile([128, 1024], F32, tag="abc")
                    nc.tensor.matmul(pc[:, 0:NT], ssel[nt], arow[:])
                    nc.tensor.matmul(pc[:, 512:512 + NT], ssel[nt], crow[:])
                    pc_sb = tmpp.tile([128, 2 * NT], BF, tag="pcsb")
                    nc.scalar.activation(
                        pc_sb[:].rearrange("p (j n) -> p j n", j=2),
                        pc[:].rearrange("p (j n) -> p j n", j=2)[:, :, 0:NT],
                        Act.Copy)
                    for ct, eng in ((0, nc.vector), (1, nc.gpsimd)):
                        t0 = tmpp.tile([128, NT], F32, tag=f"t0{ct}")
                        eng.tensor_mul(t0[:], xres[ct][:, cs], pc_sb[:, 0:NT])
                        with nc.allow_low_precision("fp8 LN output"):
                            eng.tensor_add(h3[:, ct, cs], t0[:],
                                           pc_sb[:, NT:2 * NT])

        def run_stages(it):
            run_stage1(it)
            run_stage2(it)

        # ================= stage 1: LN1 + attention =================
        def run_stage1(it):
            ctx = ExitStack()
            layer_norm("1" + it)
            lnsp = ctx.enter_context(tc.tile_pool(name=f"lnst2{it}", bufs=1,
                                                  space="PSUM"))
            apool = ctx.enter_context(tc.tile_pool(name="attn_sb", bufs=1))
            c8 = apool.tile([128, 2 * KV], F8, tag="cT8", name="cT8")
            c8v = c8[:].rearrange("p (j n) -> p j n", j=2)
            cw = apool.tile([128, KV], BF, tag="ctmp", name="ctmp")
            k_sb = [apool.tile([128, KV], BF, tag=f"k{t}", name=f"k{t}") for t in range(2)]
            v_sb = apool.tile([128, NKT * 260], BF, tag="v", name="v_sb")
            q_sb = [apool.tile([128, N], BF, tag=f"q{t}", name=f"q{t}") for t in range(2)]
            o_cat = [apool.tile([128, N], BF, tag=f"ocat{t}", name=f"ocat{t}") for t in range(2)]
            # x^2 tiles for LN2 stats, written during attention as wo lands
            sqbuf = [apool.tile([128, N], BF, tag=f"sqbuf{t}", name=f"sqbuf{t}")
                     for t in range(2)]

            with ExitStack() as pctx:
                mmp = pctx.enter_context(tc.tile_pool(name="proj_ps", bufs=3, space="PSUM"))
                # q / SR / k / v interleaved: SR runs in two row-chunks
                # (output rows 0:16 need only hbuf rows 0:32 = n-tiles 0..3),
                # so k and v for the first 4 kv-tiles start while the LN
                # applies for the tail n-tiles are still in flight.
                def emit_q(nt):
                    for mt in range(2):
                        cs = slice(nt * NT, (nt + 1) * NT)
                        ps = mmp.tile([128, NT], F32, tag="mm")
                        nc.tensor.matmul(ps[:], wq3[:, :, mt * 128:(mt + 1) * 128],
                                         h3[:, :, cs], start=True, stop=True,
                                         perf_mode=DR)
                        nc.scalar.activation(q_sb[mt][:, cs], ps[:],
                                             Act.Identity,
                                             bias=bq_sb[:, mt:mt + 1],
                                             scale=1.0 / WSC)

                def emit_sr(rlo, rhi):
                    # taps accumulate in a bf16 scratch; the last tap writes
                    # the fp8 j-major cT tile consumed by the k/v DR matmuls
                    for ct in range(2):
                        h4 = h3[:, ct, :].rearrange("p (h a w b) -> p h a w b",
                                                    a=2, b=2, h=28, w=28)
                        cw3 = cw[:].rearrange("p (h w) -> p h w", w=28)
                        c3 = c8v[:, ct, :].rearrange("p (h w) -> p h w", w=28)
                        nc.vector.tensor_scalar_mul(cw3[:, rlo:rhi, :],
                                                    h4[:, rlo:rhi, 0, :, 0],
                                                    srw_sb[ct][:, 0:1])
                        for ky, kx in ((0, 1), (1, 0), (1, 1)):
                            ti = ky * 2 + kx
                            dst = cw3 if ti != 3 else c3
                            with nc.allow_low_precision("fp8 SR output"):
                                nc.vector.scalar_tensor_tensor(
                                    dst[:, rlo:rhi, :], h4[:, rlo:rhi, ky, :, kx],
                                    srw_sb[ct][:, ti:ti + 1],
                                    cw3[:, rlo:rhi, :], op0=Alu.mult, op1=Alu.add)

                def emit_k(n0, nsz):
                    for mt in range(2):
                        ps = mmp.tile([128, NT], F32, tag="mm")
                        nc.tensor.matmul(ps[:, :nsz], wk3[:, :, mt * 128:(mt + 1) * 128],
                                         c8v[:, :, n0:n0 + nsz], start=True,
                                         stop=True, perf_mode=DR)
                        nc.vector.tensor_scalar(k_sb[mt][:, n0:n0 + nsz], ps[:, :nsz],
                                                1.0 / WSC, bk_sb[:, mt:mt + 1],
                                                op0=Alu.mult, op1=Alu.add)

                def emit_v(kts):
                    # v bias is folded into bo host-side (Wo @ bv is constant
                    # after softmax normalization), so the evict is one strided
                    # ACT copy into the ones-augmented head-packed layout
                    for kt in kts:
                        ps = mmp.tile([128, NT], F32, tag="mm")
                        nc.tensor.matmul(ps[0:KT, 0:C],
                                         c8v[:, :, kt * KT:(kt + 1) * KT],
                                         wv3[:, :, :], start=True, stop=True,
                                         perf_mode=DR)
                        vv = v_sb[0:KT, kt * 260:(kt + 1) * 260].rearrange(
                            "p (h x) -> p h x", h=NH)
                        nc.scalar.activation(vv[:, :, 0:64],
                                             ps[0:KT, 0:C].rearrange(
                                                 "p (h x) -> p h x", h=NH),
                                             Act.Identity, scale=1.0 / WSC)
                        nc.vector.memset(vv[:, :, 64:65], 1.0)

                for nt in range(4):
                    emit_q(nt)
                emit_sr(0, 16)
                emit_k(0, 448)
                emit_v(range(4))
                for nt in range(4, NNT):
                    emit_q(nt)
                emit_sr(16, 28)
                emit_k(448, 336)
                emit_v(range(4, NKT))

            for m in DW_PE_TILES:
                build_diag(f"m{m}", dw9_sb[m])

            # flash attention (heads interleaved for PE row-group packing;
            # rel-pos bias applied as exp(s)*exp(rp) with host-precomputed
            # exp(rp) multiplied in on DVE/Pool). o_cat is written raw per
            # head (Pool evict) with denominator rows batched per q-tile:
            # one DVE reciprocal over [NH, NT], PE broadcast into [128, NT]
            # PSUM per ct-tile, then one in-place DVE normalize per ct-tile.
            with ExitStack() as pctx:
                rpp = pctx.enter_context(tc.tile_pool(name="rp", bufs=3))
                ppp = pctx.enter_context(tc.tile_pool(name="pexp", bufs=3))
                sps = pctx.enter_context(tc.tile_pool(name="spsum", bufs=2, space="PSUM"))
                ops = pctx.enter_context(tc.tile_pool(name="opsum", bufs=2, space="PSUM"))
                rps = pctx.enter_context(tc.tile_pool(name="rpsum", bufs=1, space="PSUM"))
                rsp = pctx.enter_context(tc.tile_pool(name="rsb", bufs=2))
                # software-pipelined by one step: scores/exp/p-mul for item
                # i+1 are emitted before AV of item i, so the in-order PE
                # queue never parks behind an AV that waits on DVE, and the
                # exp stream (the phase bottleneck) stays fed.
                o_hold = [None, None]
                ln2_st0 = lnsp.tile([NNT, NT], F32, tag="ln2st0", name="ln2_st0")

                def emit_scores(qt, h):
                    cs = slice(qt * NT, (qt + 1) * NT)
                    ht, hr = h // 2, (h % 2) * 64
                    rp_t = rpp.tile([KT, NKT, NT], BF, name="rp_t")
                    nc.sync.dma_start(out=rp_t[:], in_=rpT_d.ap()[h, qt])
                    p_t = ppp.tile([KT, NKT, NT], BF, name="p_t")
                    p_f = p_t[:].rearrange("p a b -> p (a b)")
                    r_f = rp_t[:].rearrange("p a b -> p (a b)")
                    for g0, glen in ((0, 2), (2, 2), (4, 2), (6, 1)):
                        s_ps = sps.tile([KT, 1024], F32, name="s_ps")
                        s3v = s_ps[:].rearrange("p (a b) -> p a b", b=512)
                        for j in range(glen):
                            kt = g0 + j
                            nc.tensor.matmul(
                                s_ps[:, j * 512:j * 512 + NT],
                                k_sb[ht][hr:hr + 64, kt * KT:(kt + 1) * KT],
                                q_sb[ht][hr:hr + 64, cs], start=True, stop=True)
                        gs = slice(g0 * NT, (g0 + glen) * NT)
                        et = rsp.tile([KT, 2 * NT], BF, tag="et", name="et", bufs=3)
                        e3v = et[:].rearrange("p (a b) -> p a b", b=NT)
                        nc.scalar.activation(e3v[:, :glen, :], s3v[:, :glen, 0:NT],
                                             Act.Exp)
                        eng = nc.gpsimd if g0 >= 4 else nc.vector
                        eng.tensor_mul(p_f[:, gs], et[:, :glen * NT], r_f[:, gs])
                    return p_t

                def emit_av(qt, h, p_t):
                    cs = slice(qt * NT, (qt + 1) * NT)
                    ht, hr = h // 2, (h % 2) * 64
                    o_ps = ops.tile([65, NT], F32, name="o_ps")
                    for kt in range(NKT):
                        nc.tensor.matmul(o_ps[:],
                                         v_sb[0:KT, kt * 260 + h * 65: kt * 260 + (h + 1) * 65],
                                         p_t[:, kt, :], start=(kt == 0), stop=(kt == NKT - 1))
                    # reciprocal of the denominator row straight out of PSUM
                    # (written at partition 64, a legal matmul-rhs base); raw
                    # head output evicted into o_cat and normalized in-place
                    # per ct-tile once both heads have landed
                    rr = rsp.tile([65, NT], BF, tag="rr", name="rr", bufs=3)
                    with nc.allow_low_precision("bf16 softmax denom row"):
                        nc.vector.reciprocal(rr[64:65, :], o_ps[64:65, :])
                    nc.vector.tensor_copy(o_cat[ht][hr:hr + 64, cs], o_ps[0:64, :])
                    o_hold[h % 2] = rr
                    if h % 2 == 1:
                        ct = h // 2
                        rb_ps = rps.tile([128, NT], F32, name="rb_ps")
                        for hh in range(2):
                            nc.tensor.matmul(rb_ps[hh * 64:(hh + 1) * 64, :],
                                             onesr64[64:65, 0:64],
                                             o_hold[hh][64:65, :])
                        nc.vector.tensor_mul(o_cat[ct][:, cs], o_cat[ct][:, cs],
                                             rb_ps[:])
                    if h != NH - 1:
                        return
                    # q-tile tail: wo projection (+residual) and LN2 squares
                    for mt in range(2):
                        ps = rps.tile([128, NT], F32, name="rb_ps")
                        for kt in range(2):
                            nc.tensor.matmul(ps[:], wo_sb[kt][:, mt * 128:(mt + 1) * 128],
                                             o_cat[kt][:, cs], start=(kt == 0), stop=(kt == 1))
                        nc.vector.scalar_tensor_tensor(xres[mt][:, cs], ps[:],
                                                       bo_sb[:, mt:mt + 1],
                                                       xres[mt][:, cs],
                                                       op0=Alu.add, op1=Alu.add)
                        nc.gpsimd.tensor_mul(sqbuf[mt][:, cs], xres[mt][:, cs],
                                             xres[mt][:, cs])
                        nc.gpsimd.tensor_copy(xstage[mt][:, cs], xres[mt][:, cs])
                        # LN2 mean stats interleaved: one accumulation group
                        # spanning the whole attention, finishing with qt=6
                        nc.tensor.matmul(ln2_st0[:],
                                         zsel_b[:, NNT - 1 - qt:2 * NNT - 1 - qt],
                                         xstage[mt][:, cs],
                                         start=(qt == 0 and mt == 0),
                                         stop=(qt == NNT - 1 and mt == 1))

                pending = None
                for qt in range(NNT):
                    for h in range(NH):
                        p_t = emit_scores(qt, h)
                        if pending is not None:
                            emit_av(*pending)
                        pending = (qt, h, p_t)
                emit_av(*pending)

            layer_norm("2" + it, sq_src=sqbuf, st0_src=ln2_st0)
            ctx.close()

        # ================= stage 2: LN2 + conv-MLP + blk dwconv =================
        # dwconv inputs are x-padded to width 58 (zero cols 0 and 57) so all
        # taps are full-width and matmul outputs stay flat 2D.
        WP = WS + 2

        def run_stage2(it):
            ctx = ExitStack()
            mpool = ctx.enter_context(tc.tile_pool(name="mlp_ps", bufs=2, space="PSUM"))
            dps = ctx.enter_context(tc.tile_pool(name="dw_ps", bufs=2, space="PSUM"))
            upool = ctx.enter_context(tc.tile_pool(name="u", bufs=5))
            accp = ctx.enter_context(tc.tile_pool(name="dwacc", bufs=2))
            y2p = ctx.enter_context(tc.tile_pool(name="y2", bufs=1))
            y28 = y2p.tile([128, 8 * N], F8, tag="y28", name="y28")
            y2q = y28[:].rearrange("p (m n) -> p m n", m=8)
            y2 = [y2q[:, m, :] for m in range(8)]
            x3p = [y2p.tile([128, HS * WP], F32, tag=f"x3p{t}", name=f"x3p{t}")
                   for t in range(2)]
            x3b = [y2p.tile([128, HS * WP], BF, tag=f"x3b{t}", name=f"x3b{t}")
                   for t in range(2)]


            def tap_windows(r0):
                wins = []
                for dy, dx in TAPS:
                    rlo = max(r0, 1 if dy < 0 else 0)
                    rhi = min(r0 + 8, HS - (1 if dy > 0 else 0))
                    if rlo < rhi:
                        wins.append((dy, dx, rlo, rhi))
                return wins

            def dw_pe(src3, key, bias_col, dst):
                """3x3 depthwise conv of padded bf16 src3 [128,56,58] via PE
                diag matmuls; gelu evict with bias -> dst fp8."""
                diag = diag_all[key]
                for nt in range(NNT):
                    ps = dps.tile([128, NT], F32, name="dwps")
                    p3 = ps[:].rearrange("p (h w) -> p h w", w=WS)
                    r0 = nt * 8
                    nc.tensor.matmul(ps[:], diag[4][:], src3[:, r0:r0 + 8, 1:57],
                                     start=True, stop=False)
                    wins = tap_windows(r0)
                    for ti, (dy, dx, rlo, rhi) in enumerate(wins):
                        nc.tensor.matmul(
                            ps[:, (rlo - r0) * WS:(rhi - r0) * WS],
                            diag[tap_idx(dy, dx)][:],
                            src3[:, rlo + dy:rhi + dy, 1 + dx:57 + dx],
                            start=False, stop=(ti == len(wins) - 1))
                    with nc.allow_low_precision("fp8 y2"):
                        nc.scalar.activation(dst[:, r0 * WS:(r0 + 8) * WS], ps[:],
                                             Act.Gelu, bias=bias_col)

            def dw_pool(src3, w9_sb, bias_col, dst, r0=0, r1=HS):
                """3x3 depthwise conv rows [r0, r1) on Pool: GPSIMD has no
                per-partition-scalar ops, so each tap is mul-into-scratch +
                add-into-acc with the weight as a stride-0 broadcast AP."""
                acc = accp.tile([128, N], BF, name="acc")
                a3 = acc[:].rearrange("p (h w) -> p h w", w=WS)
                tmp = accp.tile([128, N], BF, name="ptmp", tag="ptmp", bufs=2)
                t3 = tmp[:].rearrange("p (h w) -> p h w", w=WS)
                with nc.allow_low_precision("bf16 dwconv accumulator"):
                    wb = w9_sb[:, 4:5].broadcast_to([128, r1 - r0, WS])
                    nc.gpsimd.tensor_mul(a3[:, r0:r1, :], src3[:, r0:r1, 1:57], wb)
                    for dy, dx in TAPS:
                        rlo = max(r0, 1 if dy < 0 else 0)
                        rhi = min(r1, HS - (1 if dy > 0 else 0))
                        t = tap_idx(dy, dx)
                        wb = w9_sb[:, t:t + 1].broadcast_to([128, rhi - rlo, WS])
                        nc.gpsimd.tensor_mul(
                            t3[:, rlo:rhi, :],
                            src3[:, rlo + dy:rhi + dy, 1 + dx:57 + dx], wb)
                        nc.gpsimd.tensor_add(a3[:, rlo:rhi, :], a3[:, rlo:rhi, :],
                                             t3[:, rlo:rhi, :])
                with nc.allow_low_precision("fp8 y2"):
                    nc.scalar.activation(dst[:, r0 * WS:r1 * WS],
                                         acc[:, r0 * WS:r1 * WS], Act.Gelu,
                                         bias=bias_col)

            def dw_dve(src3, w9_sb, bias_col, dst, eng, r0=0, r1=HS):
                """3x3 depthwise conv rows [r0, r1) on DVE or Pool: center-tap
                init (+bias), 8 stt taps. bf16 accumulator keeps DVE in the 2x
                16-bit mode."""
                acc = accp.tile([128, N], BF, name="acc")
                a3 = acc[:].rearrange("p (h w) -> p h w", w=WS)
                with nc.allow_low_precision("bf16 dwconv accumulator"):
                    eng.tensor_scalar(a3[:, r0:r1, :], src3[:, r0:r1, 1:57],
                                      w9_sb[:, 4:5],
                                      bias_col, op0=Alu.mult, op1=Alu.add)
                    for dy, dx in TAPS:
                        rlo = max(r0, 1 if dy < 0 else 0)
                        rhi = min(r1, HS - (1 if dy > 0 else 0))
                        t = tap_idx(dy, dx)
                        eng.scalar_tensor_tensor(
                            a3[:, rlo:rhi, :], src3[:, rlo + dy:rhi + dy, 1 + dx:57 + dx],
                            w9_sb[:, t:t + 1], a3[:, rlo:rhi, :], op0=Alu.mult, op1=Alu.add)
                with nc.allow_low_precision("fp8 y2"):
                    nc.scalar.activation(dst[:, r0 * WS:r1 * WS],
                                         acc[:, r0 * WS:r1 * WS], Act.Gelu)

            # emit all w1 projections first so the PE stream isn't blocked
            # behind per-tile dwconvs; slow-engine (Pool/DVE) dw tiles come
            # first so their long serial tap chains start as early as
            # possible, and each runs as two row-halves so w2 for the first
            # row blocks can start while second halves are still in flight
            u3s = {}
            pad1 = onespad[:].rearrange("p (h o) -> p h o", o=1)
            slow = [m for m in range(8) if m not in DW_PE_TILES]
            for m in slow + list(DW_PE_TILES):
                u = upool.tile([128, HS * WP], BF, name="u")
                u3 = u[:].rearrange("p (h w) -> p h w", w=WP)
                u3s[m] = u3
                nc.vector.tensor_scalar_mul(u3[:, :, 0:1], pad1, ncol_sb[:, m:m + 1])
                nc.vector.tensor_scalar_mul(u3[:, :, 57:58], pad1, ncol_sb[:, m:m + 1])
                for nt0 in range(0, NNT, 2):
                    npair = min(2, NNT - nt0)
                    ps = mpool.tile([128, 1024], F32, tag="mm", name="mmps")
                    for j in range(npair):
                        cs = slice((nt0 + j) * NT, (nt0 + j + 1) * NT)
                        nc.tensor.matmul(ps[:, j * 512:j * 512 + NT],
                                         w13[:, :, m * 128:(m + 1) * 128],
                                         h3[:, :, cs], start=True, stop=True,
                                         perf_mode=DR)
                    psv = ps[:].rearrange("p (j n) -> p j n", j=2)[:, 0:npair, 0:NT]
                    nc.scalar.activation(
                        u3[:, nt0 * 8:(nt0 + npair) * 8, 1:57], psv,
                        Act.Gelu, bias=b1_sb[:, m:m + 1], scale=1.0 / WSC)
            RSPLIT = 32
            for half in ((0, RSPLIT), (RSPLIT, HS)):
                for m in slow:
                    if m in DW_POOL_TILES:
                        dw_pool(u3s[m], dw9_sb[m], dwb_sb[:, m:m + 1], y2[m],
                                r0=half[0], r1=half[1])
                    else:
                        dw_dve(u3s[m], dw9_sb[m], dwb_sb[:, m:m + 1], y2[m],
                               nc.vector, r0=half[0], r1=half[1])
            for m in DW_PE_TILES:
                dw_pe(u3s[m], f"m{m}", dwb_sb[:, m:m + 1], y2[m])

            # diag matrices for the PE blk tiles (DVE/ACT, cheap, deps ready)
            for ct in range(2):
                if ct not in BLK_SLOW_TILES:
                    build_diag(f"b{ct}", bk9_sb[ct])

            # final blk dwconv row-block: Pool tiles accumulate 8 taps from
            # the exact f32 xp3 (bf16 acc); PE tiles run diag matmuls over
            # xb3. Exact-fp32 center/residual fused in the evict:
            # f = acc + (1 + w_center) * x3.
            def emit_blk_nt(ct, nt):
                xp3 = x3p[ct][:].rearrange("p (h w) -> p h w", w=WP)
                r0 = nt * 8
                fo = accp.tile([128, NT], F32, tag="fout", name="fout", bufs=3)
                f3 = fo[:].rearrange("p (h w) -> p h w", w=WS)
                if ct in BLK_SLOW_TILES:
                    acc = accp.tile([128, NT], BF, tag="blkacc", name="blkacc",
                                    bufs=2)
                    a3 = acc[:].rearrange("p (h w) -> p h w", w=WS)
                    with nc.allow_low_precision("bf16 blk dwconv accumulator"):
                        nc.vector.tensor_scalar(
                            a3[:, :, :], xp3[:, r0:r0 + 8, 0:56],
                            bk9_sb[ct][:, 3:4], bkb_col[:, ct:ct + 1],
                            op0=Alu.mult, op1=Alu.add)
                        for dy, dx in TAPS:
                            if (dy, dx) == (0, -1):
                                continue
                            rlo = max(r0, 1 if dy < 0 else 0)
                            rhi = min(r0 + 8, HS - (1 if dy > 0 else 0))
                            t = tap_idx(dy, dx)
                            nc.vector.scalar_tensor_tensor(
                                a3[:, rlo - r0:rhi - r0, :],
                                xp3[:, rlo + dy:rhi + dy, 1 + dx:57 + dx],
                                bk9_sb[ct][:, t:t + 1],
                                a3[:, rlo - r0:rhi - r0, :],
                                op0=Alu.mult, op1=Alu.add)
                    nc.vector.scalar_tensor_tensor(
                        f3[:, :, :], xp3[:, r0:r0 + 8, 1:57], bk9_sb[ct][:, 4:5],
                        a3[:, :, :], op0=Alu.mult, op1=Alu.add)
                else:
                    diag = diag_all[f"b{ct}"]
                    xb3 = x3b[ct][:].rearrange("p (h w) -> p h w", w=WP)
                    ps = dps.tile([128, NT], F32, name="blkps")
                    p3 = ps[:].rearrange("p (h w) -> p h w", w=WS)
                    nc.tensor.matmul(ps[:], bkb_row[0:1, ct * 128:(ct + 1) * 128],
                                     onesn[:], start=True, stop=False)
                    wins = tap_windows(r0)
                    for ti, (dy, dx, rlo, rhi) in enumerate(wins):
                        dg = diag[tap_idx(dy, dx)]
                        nc.tensor.matmul(
                            p3[:, rlo - r0:rhi - r0, :], dg[:],
                            xb3[:, rlo + dy:rhi + dy, 1 + dx:57 + dx],
                            start=False, stop=(ti == len(wins) - 1))
                    nc.vector.scalar_tensor_tensor(
                        f3[:, :, :], xp3[:, r0:r0 + 8, 1:57], bk9_sb[ct][:, 4:5],
                        ps[:].rearrange("p (h w) -> p h w", w=WS),
                        op0=Alu.mult, op1=Alu.add)
                nc.sync.dma_start(
                    out=fT_d[ct * 128:(ct + 1) * 128, r0 * WS:(r0 + 8) * WS],
                    in_=fo[:])

            # w2 (+bn2/pbn folded bias) + residual -> x3p (padded, f32), bf16
            # copy only for PE blk tiles; blkdw for row-block nt-1 is emitted
            # right after w2's nt lands so the final conv pipelines with w2
            # instead of forming a serial tail.
            for mt in range(2):
                xp3 = x3p[mt][:].rearrange("p (h w) -> p h w", w=WP)
                nc.vector.memset(xp3[:, :, 0:1], 0.0)
                nc.vector.memset(xp3[:, :, 57:58], 0.0)
                if mt not in BLK_SLOW_TILES:
                    xb3 = x3b[mt][:].rearrange("p (h w) -> p h w", w=WP)
                    nc.vector.memset(xb3[:, :, 0:1], 0.0)
                    nc.vector.memset(xb3[:, :, 57:58], 0.0)
            for nt in range(NNT):
                for mt in range(2):
                    xp3 = x3p[mt][:].rearrange("p (h w) -> p h w", w=WP)
                    cs = slice(nt * NT, (nt + 1) * NT)
                    ps = mpool.tile([128, NT], F32, tag="mm", name="mmps2")
                    # bias rides the accumulation as a K=1 ones matmul (the
                    # 1/32 weight descale in the evict leaves no slot for it)
                    nc.tensor.matmul(ps[:], b2rr[0:1, mt * 128:(mt + 1) * 128],
                                     onesn[:], start=True, stop=False,
                                     skip_group_check=True)
                    for q in range(4):
                        nc.tensor.matmul(ps[:], w23[:, q, :, mt * 128:(mt + 1) * 128],
                                         y2q[:, 2 * q:2 * q + 2, cs], start=False,
                                         stop=(q == 3), perf_mode=DR,
                                         skip_group_check=True)
                    nc.vector.scalar_tensor_tensor(
                        xp3[:, nt * 8:(nt + 1) * 8, 1:57], ps[:], 1.0 / WSC,
                        xres[mt][:, cs], op0=Alu.mult, op1=Alu.add)
                    if mt not in BLK_SLOW_TILES:
                        xb3 = x3b[mt][:].rearrange("p (h w) -> p h w", w=WP)
                        nc.gpsimd.tensor_copy(xb3[:, nt * 8:(nt + 1) * 8, 1:57],
                                              xp3[:, nt * 8:(nt + 1) * 8, 1:57])
                if nt >= 1:
                    for ct in range(2):
                        emit_blk_nt(ct, nt - 1)
            for ct in range(2):
                emit_blk_nt(ct, NNT - 1)
            ctx.close()

        for it in range(iters):
            body(f"_i{it}")

    nc.compile()
    return nc


_CACHE = {}


def _get_program():
    if "nc" not in _CACHE:
        _CACHE["nc"] = _build_program()
    return _CACHE["nc"]


def _prep_inputs(inputs):
    f64 = np.float64
    g1 = inputs["ln1_g"].astype(f64); b1ln = inputs["ln1_b"].astype(f64)
    g2 = inputs["ln2_g"].astype(f64); b2ln = inputs["ln2_b"].astype(f64)
    scale = DH ** -0.5

    def bn_ac(g, b, m, v):
        a = np.asarray(g, f64) / np.sqrt(np.asarray(v, f64) + EPS)
        return a, np.asarray(b, f64) - np.asarray(m, f64) * a

    wq = np.asarray(inputs["wq"], f64); wk = np.asarray(inputs["wk"], f64)
    wv = np.asarray(inputs["wv"], f64); wo = np.asarray(inputs["wo"], f64)

    wq_eff = wq * g1[None, :] * scale
    bq_eff = (wq @ b1ln + np.asarray(inputs["bq"], f64)) * scale

    sa, sc = bn_ac(inputs["srbn_g"], inputs["srbn_b"], inputs["srbn_m"], inputs["srbn_v"])
    srw4 = np.asarray(inputs["sr_w"], f64).reshape(C, 4)  # [c, ky*2+kx]
    srw_eff = srw4 * (g1 * sa)[:, None]
    d_const = sa * (b1ln * srw4.sum(1) + np.asarray(inputs["sr_b"], f64)) + sc
    bk_eff = wk @ d_const + np.asarray(inputs["bk"], f64)
    bv_eff = wv @ d_const + np.asarray(inputs["bv"], f64)
    bo_eff = np.asarray(inputs["bo"], f64) + wo @ bv_eff

    w1 = np.asarray(inputs["w1"], f64)
    w1_eff = w1 * g2[None, :]
    b1_eff = w1 @ b2ln + np.asarray(inputs["b1"], f64)
    a1_, c1_ = bn_ac(inputs["bn1_g"], inputs["bn1_b"], inputs["bn1_m"], inputs["bn1_v"])

    dw9 = np.asarray(inputs["dw_w"], f64).reshape(HID, 9).copy()
    dw9[:, 4] += 1.0  # residual fold
    # bn1 fold: dw(a1*u + c1) + dwb = (dw9*a1)(u) + (dwb + c1*sum(dw9)) with
    # u padded by -c1/a1 so the conv's zero padding stays exact at borders
    dwb = np.asarray(inputs["dw_b"], f64) + c1_ * dw9.sum(1)
    ncol = -c1_ / a1_
    dw9 = dw9 * a1_[:, None]

    pa, pc = bn_ac(inputs["pbn_g"], inputs["pbn_b"], inputs["pbn_m"], inputs["pbn_v"])
    a2_, c2_ = bn_ac(inputs["bn2_g"], inputs["bn2_b"], inputs["bn2_m"], inputs["bn2_v"])
    w2 = np.asarray(inputs["w2"], f64)
    w2_eff = (w2 * pa[None, :]) * a2_[:, None]
    b2_eff = a2_ * (w2 @ pc + np.asarray(inputs["b2"], f64)) + c2_

    bk9 = np.asarray(inputs["blkdw_w"], f64).reshape(C, 9).copy()
    bk9[:, 4] += 1.0
    bkb = np.asarray(inputs["blkdw_b"], f64)

    bf = lambda a: np.ascontiguousarray(np.asarray(a, np.float32)).astype(BF16)
    f32 = lambda a: np.ascontiguousarray(np.asarray(a, np.float32))
    WSC = 32.0
    F8NP = ml_dtypes.float8_e4m3

    def f8w2(a):  # [256, M] -> j-major [128, 2, M] fp8, pre-scaled x32
        a = np.asarray(a, f64) * WSC
        a = np.clip(a, -224.0, 224.0).astype(np.float32)
        return np.ascontiguousarray(
            a.reshape(2, 128, -1).transpose(1, 0, 2).reshape(128, -1)).astype(F8NP)

    def f8w8(a):  # [1024, M] -> [128, q=4, j=2, M] fp8, pre-scaled x32
        a = np.asarray(a, f64) * WSC
        a = np.clip(a, -224.0, 224.0).astype(np.float32)
        M = a.shape[1]
        return np.ascontiguousarray(
            a.reshape(4, 2, 128, M).transpose(2, 0, 1, 3).reshape(128, -1)).astype(F8NP)

    shared = {
        "rpT": np.ascontiguousarray(
            np.exp(np.asarray(inputs["relative_pos"], np.float64))
            .reshape(NH, NNT, NT, NKT, KT).transpose(0, 1, 4, 3, 2)).astype(BF16),
        "wq8": f8w2(wq_eff.T), "wk8": f8w2(wk.T), "wv8": f8w2(wv.T),
        "woT": bf(wo.T),
        "w18": f8w2(w1_eff.T), "w28": f8w8(w2_eff.T),
        "b2rr": bf(WSC * b2_eff[None, :]),
        "bq": f32(bq_eff), "bk": f32(bk_eff),
        "bo": f32(bo_eff), "b1": f32(b1_eff), "ncol": f32(ncol),
        "srw": f32(srw_eff), "dw9": f32(dw9), "dwb": f32(dwb),
        "bk9": f32(bk9), "bkb": bf(bkb[None, :]), "bkbc": f32(bkb),
        "eyeb": np.eye(128, dtype=np.float32).astype(BF16),
        "onesr": np.ones((1, 128), np.float32).astype(BF16),
        "sselw": np.repeat(np.eye(NNT, dtype=np.float32), 128, axis=1).astype(BF16),
    }
    x = np.asarray(inputs["x"], np.float32)
    in_maps = []
    for b in range(B):
        m = dict(shared)
        m["xT"] = np.ascontiguousarray(x[b].T)
        in_maps.append(m)
    return in_maps


def kernel(**inputs):
    from concourse.bass_utils import run_bass_kernel_spmd
    nc = _get_program()
    in_maps = _prep_inputs(inputs)
    res = run_bass_kernel_spmd(nc, in_maps, core_ids=list(range(B)))
    out = np.stack([res.results[b]["fT"].T for b in range(B)], axis=0)
    return np.ascontiguousarray(out, dtype=np.float32)



# revision 89
# speedup vs baseline: 11.9098x; 11.9098x over previous
"""Fused PVT-style transformer block kernel for Trainium2 (8 NeuronCores).

Sharding: pure data-parallel over batch B=8 -> one batch item per core.
Layout: channel-major ("transposed") activations [C(part), N(free)] throughout;
host pre-transposes x and relative_pos, post-transposes the output.

Per-core pipeline (N=3136=56x56 tokens, C=256, 4 heads x 64, KV=784=28x28,
HID=1024):
  LN1 (PE ones-matmul stats + PE K=1 broadcast + DVE apply; gamma/beta folded
  into downstream weights) -> q/k/v projections (bf16 PE) with the 2x2/s2
  spatial-reduction depthwise conv on DVE -> flash attention per (head,
  q-tile): scores^T = k^T.T @ q^T with rel-pos bias added via identity matmul
  into PSUM, exp on ACT (no max-subtraction: logits are O(1)), AV matmul with
  ones-row-augmented V giving the softmax denominator for free -> wo
  projection + residual -> LN2 -> conv1x1 (+gelu+bn1) -> 3x3 depthwise conv
  split across PE (fp32r diagonal matmuls into PSUM) and DVE (fused
  scalar_tensor_tensor taps) -> gelu -> conv1x1 (bn2/pbn folded) + residual ->
  final 3x3 depthwise conv (residual folded into center tap) -> output.
"""

import numpy as np
import ml_dtypes

B, N, C, NH, DH, KV, HID = 8, 3136, 256, 4, 64, 784, 1024
HS = WS = 56
NT = 448            # n-tile (8 rows of 56)
NNT = N // NT       # 7
KT = 112            # kv tile
NKT = KV // KT      # 7
EPS = 1e-5
BF16 = ml_dtypes.bfloat16

DW_PE_TILES = (3, 4, 5, 6, 7)   # HID ch-tiles whose dwconv runs on PE
DW_POOL_TILES = (1,)            # dw tiles on Pool via mul+add TT pairs
BLK_SLOW_TILES = ()             # blk dwconv ch-tiles on DVE (rest: PE)

TAPS = [(dy, dx) for dy in (-1, 0, 1) for dx in (-1, 0, 1) if (dy, dx) != (0, 0)]


def tap_idx(dy, dx):
    return (dy + 1) * 3 + (dx + 1)


def _build_program(iters=1, feedback=False):
    """feedback=True: body i>0 reads its x from fT (previous body's output)
    instead of xT, forcing a true serial dependency chain across bodies —
    used only for timing (defeats any cross-body dead-code elimination)."""
    import concourse.bacc as bacc
    import concourse.mybir as mybir
    import concourse.tile as tile
    from contextlib import ExitStack

    dt = mybir.dt
    F32, BF, F32R = dt.float32, dt.bfloat16, dt.float32r
    Alu = mybir.AluOpType
    Act = mybir.ActivationFunctionType
    DR = mybir.MatmulPerfMode.DoubleRow
    WSC = 32.0   # host-side fp8 weight pre-scale, undone at evict
    I32 = dt.int32
    FE_A = float(2 ** 23 / np.log(2.0))      # Schraudolph fast-exp scale
    FE_B = float(127 * 2 ** 23 - 486411)     # bias, minimax-centered

    nc = bacc.Bacc("TRN2", target_bir_lowering=False, debug=False, num_devices=8)

    def din(name, shape, dtype):
        return nc.dram_tensor(name, shape, dtype, kind="ExternalInput")

    F8 = dt.float8e4

    xT_d = din("xT", [C, N], F32)
    rpT_d = din("rpT", [NH, NNT, KT, NKT, NT], BF)
    # fp8 projection weights, pre-scaled x32 host-side (de-scaled at evict)
    # and laid out j-major for DoubleRow: [128, j=2, out] with j the
    # contraction half (channels 128j..128j+127).
    wq8_d = din("wq8", [128, 2 * C], F8)
    wk8_d = din("wk8", [128, 2 * C], F8)
    wv8_d = din("wv8", [128, 2 * C], F8)
    woT_d = din("woT", [C, C], BF)
    w18_d = din("w18", [128, 2 * HID], F8)
    w28_d = din("w28", [128, 8 * C], F8)
    b2rr_d = din("b2rr", [1, C], BF)
    bq_d = din("bq", [C], F32)
    bk_d = din("bk", [C], F32)
    bo_d = din("bo", [C], F32)
    b1_d = din("b1", [HID], F32)
    ncol_d = din("ncol", [HID], F32)
    srw_d = din("srw", [C, 4], F32)
    dw9_d = din("dw9", [HID, 9], F32)
    dwb_d = din("dwb", [HID], F32)
    bk9_d = din("bk9", [C, 9], F32)
    bkb_d = din("bkb", [1, C], BF)
    bkbc_d = din("bkbc", [C], F32)
    eyeb_d = din("eyeb", [128, 128], BF)
    sselw_d = din("sselw", [NNT, NNT * 128], BF)
    fT_d = nc.dram_tensor("fT", [C, N], F32, kind="ExternalOutput")

    def r32(ap):
        return ap.bitcast(F32R)

    with tile.TileContext(nc) as tc, ExitStack() as octx:
        wpool = octx.enter_context(tc.tile_pool(name="weights", bufs=1))
        persist = octx.enter_context(tc.tile_pool(name="persist", bufs=1))
        digp = octx.enter_context(tc.tile_pool(name="diag", bufs=1))

        # input tiles first: LN1's first stats matmul needs xres[*][:, :448];
        # emitting these DMAs before the ~1.7MB of weight loads removes the
        # 29us startup stall (weights aren't needed until q/k/v projections).
        xres = [persist.tile([128, N], F32, tag=f"xres{t}", name=f"xres{t}") for t in range(2)]
        for nt in range(NNT):
            cs = slice(nt * NT, (nt + 1) * NT)
            for t in range(2):
                nc.sync.dma_start(out=xres[t][:, cs], in_=xT_d[t * 128:(t + 1) * 128, cs])

        def wload(dram_ap, shape, dtype, tag):
            t = wpool.tile(shape, dtype, tag=tag, name=tag)
            nc.sync.dma_start(out=t[:], in_=dram_ap)
            return t

        # LN1's broadcast matmuls need ssel before any other weight arrives
        ssel_all = wload(sselw_d[:, :], [NNT, NNT * 128], BF, "sselw")
        ssel = [ssel_all[:, j * 128:(j + 1) * 128] for j in range(NNT)]
        wq3 = wload(wq8_d[:, :], [128, 2 * C], F8, "wq8")[:].rearrange(
            "p (j m) -> p j m", j=2)
        wk3 = wload(wk8_d[:, :], [128, 2 * C], F8, "wk8")[:].rearrange(
            "p (j m) -> p j m", j=2)
        wv3 = wload(wv8_d[:, :], [128, 2 * C], F8, "wv8")[:].rearrange(
            "p (j m) -> p j m", j=2)
        wo_sb = [wload(woT_d[k * 128:(k + 1) * 128, :], [128, C], BF, f"wo{k}") for k in range(2)]
        w13 = wload(w18_d[:, :], [128, 2 * HID], F8, "w18")[:].rearrange(
            "p (j m) -> p j m", j=2)
        w23 = wload(w28_d[:, :], [128, 8 * C], F8, "w28")[:].rearrange(
            "p (q j m) -> p q j m", q=4, j=2)
        b2rr = wload(b2rr_d[:, :], [1, C], BF, "b2rr")
        eyeb = wload(eyeb_d[:, :], [128, 128], BF, "eyeb")
        srw_sb = [wload(srw_d.ap().rearrange("(t p) k -> p t k", p=128)[:, t, :],
                        [128, 4], F32, f"srw{t}") for t in range(2)]
        dw9_sb = [wload(dw9_d.ap().rearrange("(t p) k -> p t k", p=128)[:, t, :],
                        [128, 9], F32, f"dw9_{t}") for t in range(8)]
        bk9_sb = [wload(bk9_d.ap().rearrange("(t p) k -> p t k", p=128)[:, t, :],
                        [128, 9], F32, f"bk9_{t}") for t in range(2)]

        def vload(dram, n, tag):
            t = wpool.tile([128, n // 128], F32, tag=tag, name=tag)
            nc.sync.dma_start(out=t[:], in_=dram.ap().rearrange("(t p) -> p t", p=128))
            return t

        bq_sb = vload(bq_d, C, "bq")
        bk_sb = vload(bk_d, C, "bk")
        bo_sb = vload(bo_d, C, "bo")
        b1_sb = vload(b1_d, HID, "b1")
        ncol_sb = vload(ncol_d, HID, "ncol")
        dwb_sb = vload(dwb_d, HID, "dwb")
        bkb_row = wload(bkb_d[:, :], [1, C], BF, "bkb")
        bkb_col = vload(bkbc_d, C, "bkbc")
        onesn = wpool.tile([1, NT], BF, tag="onesn")
        nc.vector.memset(onesn[:], 1.0)
        # head-pair selector rows at partitions 64/65: one matmul broadcasts
        # both heads' reciprocal denominator rows into a [128, NT] tile
        sel2 = wpool.tile([97, 128], BF, tag="sel2")
        nc.vector.memset(sel2[64:97, :], 0.0)
        nc.vector.memset(sel2[64:65, 0:64], 1.0)
        nc.vector.memset(sel2[96:97, 64:128], 1.0)
        # ones-selector windows: zsel[:, 6-nt:13-nt] is [128, NNT] with ones in
        # column nt -> stats matmul writes partition nt of a [NNT, NT] PSUM
        # tile (matmul out base partition must be 0/32/64, so row-packing goes
        # through the stationary operand instead).
        zsel_b = wpool.tile([128, 2 * NNT - 1], BF, tag="zsel_b")
        nc.vector.memset(zsel_b[:], 0.0)
        nc.vector.memset(zsel_b[:, NNT - 1:NNT], 1.0)
        onespad = wpool.tile([128, HS], BF, tag="onespad")
        nc.vector.memset(onespad[:], 1.0)
        epscol = wpool.tile([128, 1], F32, tag="epscol")
        nc.vector.memset(epscol[:], EPS)

        # diag matrices for the PE dwconv paths; built during the qkv
        # window (ACT/DVE idle there) so the MLP never stalls on them
        diag_all = {}

        def build_diag(key, w9_sb):
            diag = []
            for t in range(9):
                dg = digp.tile([128, 128], BF, tag=f"dg{key}_{t}",
                               name=f"dg{key}_{t}")
                if t % 2 == 0:
                    nc.vector.tensor_scalar_mul(dg[:], eyeb[:], w9_sb[:, t:t + 1])
                else:
                    nc.scalar.activation(dg[:], eyeb[:], Act.Identity,
                                         bias=0.0, scale=w9_sb[:, t:t + 1])
                diag.append(dg)
            diag_all[key] = diag

        # LN output (fp8, un-affine'd), j-major [p, ct, n] for DoubleRow
        # moving operands; reused for LN1 and LN2
        h8 = persist.tile([128, 2 * N], F8, tag="h8", name="h8")
        h3 = h8[:].rearrange("p (j n) -> p j n", j=2)
        # bf16 staging copies of x so LN stats matmuls run at bf16 rate
        # (the BIR verifier rejects f32r fed straight from a DMA)
        xstage = [persist.tile([128, N], BF, tag=f"xstg{t}", name=f"xstg{t}")
                  for t in range(2)]


        def body(suffix):
            if not suffix.endswith("i0"):
                src_d = fT_d if feedback else xT_d
                for t in range(2):
                    for nt in range(NNT):
                        cs = slice(nt * NT, (nt + 1) * NT)
                        nc.sync.dma_start(out=xres[t][:, cs],
                                          in_=src_d[t * 128:(t + 1) * 128, cs])
            run_stages(suffix)

        def layer_norm(suffix, sq_src=None, st0_src=None):
            """hbuf <- (xres - mean_c) * rsqrt(var_c + eps).

            Stats land in partition-packed [NNT, NT] PSUM tiles (one partition
            per n-tile), so the per-row pipeline (square/var/sqrt/recip) runs
            once over [NNT, NT] instead of NNT times over [1, NT]. Stats
            matmuls use f32r (1 cyc/row at >=256 moving vs 4 for fp32);
            squares run on the otherwise-idle Pool engine."""
            with ExitStack() as ctx:
                sqp = ctx.enter_context(tc.tile_pool(name=f"ln_sq{suffix}", bufs=2))
                stp = ctx.enter_context(tc.tile_pool(name=f"ln_st{suffix}", bufs=1, space="PSUM"))
                bcp = ctx.enter_context(tc.tile_pool(name=f"ln_bc{suffix}", bufs=2, space="PSUM"))
                rowp = ctx.enter_context(tc.tile_pool(name=f"ln_row{suffix}", bufs=1))
                tmpp = ctx.enter_context(tc.tile_pool(name=f"ln_tmp{suffix}", bufs=3))
                st0 = st0_src if st0_src is not None else stp.tile(
                    [NNT, NT], F32, tag="st0", name="st0")
                st1 = stp.tile([NNT, NT], F32, tag="st1", name="st1")
                for nt in range(NNT):
                    cs = slice(nt * NT, (nt + 1) * NT)
                    for ct in range(2):
                        if st0_src is not None:
                            continue
                        nc.gpsimd.tensor_copy(xstage[ct][:, cs], xres[ct][:, cs])
                        nc.tensor.matmul(st0[:], zsel_b[:, NNT - 1 - nt:2 * NNT - 1 - nt],
                                         xstage[ct][:, cs],
                                         start=(nt == 0 and ct == 0),
                                         stop=(nt == NNT - 1 and ct == 1))
                    for ct in range(2):
                        if sq_src is None:
                            sq = sqp.tile([128, NT], BF)
                            k3 = (2 * nt + ct) % 3
                            if k3 == 0:
                                nc.scalar.square(sq[:], xstage[ct][:, cs])
                            else:
                                eng = nc.vector if k3 == 1 else nc.gpsimd
                                eng.tensor_mul(sq[:], xstage[ct][:, cs],
                                               xstage[ct][:, cs])
                            sqv = sq[:]
                        else:
                            sqv = sq_src[ct][:, cs]
                        nc.tensor.matmul(st1[:], zsel_b[:, NNT - 1 - nt:2 * NNT - 1 - nt],
                                         sqv,
                                         start=(nt == 0 and ct == 0),
                                         stop=(nt == NNT - 1 and ct == 1))
                m2 = rowp.tile([NNT, NT], F32, tag="m2", name="m2")
                nc.scalar.activation(m2[:], st0[:], Act.Square, scale=1.0 / C)
                var = rowp.tile([NNT, NT], F32, tag="var", name="var")
                nc.vector.scalar_tensor_tensor(var[:], st1[:], 1.0 / C, m2[:],
                                               op0=Alu.mult, op1=Alu.subtract)
                # rstd = rsqrt(var+eps) via the bit-trick + one Newton step,
                # all on DVE: avoids Ln/Sqrt ACT ops entirely, so the only
                # ACT table sets a body touches are Exp (attention) and Gelu
                # (MLP) - two table loads per body instead of six.
                vpe = rowp.tile([NNT, NT], F32, tag="vpe", name="vpe")
                nc.vector.tensor_scalar(vpe[:], var[:], EPS, None, op0=Alu.add)
                t2i = rowp.tile([NNT, NT], I32, tag="t2i", name="t2i")
                nc.vector.tensor_scalar(t2i[:], vpe[:].bitcast(I32), 1, None,
                                        op0=Alu.logical_shift_right)
                y0i = rowp.tile([NNT, NT], I32, tag="y0i", name="y0i")
                nc.vector.tensor_scalar(y0i[:], t2i[:], -1.0, float(0x5F3759DF),
                                        op0=Alu.mult, op1=Alu.add)
                y0f = y0i[:].bitcast(F32)
                zz = rowp.tile([NNT, NT], F32, tag="zz", name="zz")
                nc.vector.tensor_mul(zz[:], y0f, y0f)
                nc.vector.tensor_mul(zz[:], zz[:], vpe[:])
                nc.vector.tensor_scalar(zz[:], zz[:], -0.5, 1.5,
                                        op0=Alu.mult, op1=Alu.add)
                arow = rowp.tile([NNT, NT], BF, tag="arow", name="arow")
                with nc.allow_low_precision("bf16 rstd broadcast row"):
                    nc.vector.tensor_mul(arow[:], y0f, zz[:])
                crow = rowp.tile([NNT, NT], BF, tag="crow", name="crow")
                nc.vector.scalar_tensor_tensor(crow[:], st0[:], -1.0 / C, arow[:],
                                               op0=Alu.mult, op1=Alu.mult)
                for nt in range(NNT):
                    cs = slice(nt * NT, (nt + 1) * NT)
                    pc = bcp.tile([128, 1024], F32, tag="abc")
                    nc.tensor.matmul(pc[:, 0:NT], ssel[nt], arow[:])
                    nc.tensor.matmul(pc[:, 512:512 + NT], ssel[nt], crow[:])
                    pc_sb = tmpp.tile([128, 2 * NT], BF, tag="pcsb")
                    nc.scalar.activation(
                        pc_sb[:].rearrange("p (j n) -> p j n", j=2),
                        pc[:].rearrange("p (j n) -> p j n", j=2)[:, :, 0:NT],
                        Act.Copy)
                    for ct, eng in ((0, nc.vector), (1, nc.gpsimd)):
                        t0 = tmpp.tile([128, NT], F32, tag=f"t0{ct}")
                        eng.tensor_mul(t0[:], xres[ct][:, cs], pc_sb[:, 0:NT])
                        with nc.allow_low_precision("fp8 LN output"):
                            eng.tensor_add(h3[:, ct, cs], t0[:],
                                           pc_sb[:, NT:2 * NT])

        def run_stages(it):
            run_stage1(it)
            run_stage2(it)

        # ================= stage 1: LN1 + attention =================
        def run_stage1(it):
            ctx = ExitStack()
            layer_norm("1" + it)
            lnsp = ctx.enter_context(tc.tile_pool(name=f"lnst2{it}", bufs=1,
                                                  space="PSUM"))
            apool = ctx.enter_context(tc.tile_pool(name="attn_sb", bufs=1))
            c8 = apool.tile([128, 2 * KV], F8, tag="cT8", name="cT8")
            q8 = [apool.tile([32, 2 * N], F8, tag=f"q8_{h}", name=f"q8_{h}")
                  for h in range(NH)]
            k8 = [apool.tile([32, 2 * KV], F8, tag=f"k8_{h}", name=f"k8_{h}")
                  for h in range(NH)]
            q8v = [t[:].rearrange("p (j n) -> p j n", j=2) for t in q8]
            k8v = [t[:].rearrange("p (j n) -> p j n", j=2) for t in k8]
            c8v = c8[:].rearrange("p (j n) -> p j n", j=2)
            cw = apool.tile([128, KV], BF, tag="ctmp", name="ctmp")
            k_sb = [apool.tile([128, KV], BF, tag=f"k{t}", name=f"k{t}") for t in range(2)]
            v_sb = apool.tile([128, NKT * 260], BF, tag="v", name="v_sb")
            q_sb = [apool.tile([128, N], BF, tag=f"q{t}", name=f"q{t}") for t in range(2)]
            o_cat = [apool.tile([128, N], BF, tag=f"ocat{t}", name=f"ocat{t}") for t in range(2)]
            # x^2 tiles for LN2 stats, written during attention as wo lands
            sqbuf = [apool.tile([128, N], BF, tag=f"sqbuf{t}", name=f"sqbuf{t}")
                     for t in range(2)]

            with ExitStack() as pctx:
                mmp = pctx.enter_context(tc.tile_pool(name="proj_ps", bufs=3, space="PSUM"))
                # q / SR / k / v interleaved: SR runs in two row-chunks
                # (output rows 0:16 need only hbuf rows 0:32 = n-tiles 0..3),
                # so k and v for the first 4 kv-tiles start while the LN
                # applies for the tail n-tiles are still in flight.
                def emit_q(nt):
                    for mt in range(2):
                        cs = slice(nt * NT, (nt + 1) * NT)
                        ps = mmp.tile([128, NT], F32, tag="mm")
                        nc.tensor.matmul(ps[:], wq3[:, :, mt * 128:(mt + 1) * 128],
                                         h3[:, :, cs], start=True, stop=True,
                                         perf_mode=DR)
                        nc.scalar.activation(q_sb[mt][:, cs], ps[:],
                                             Act.Identity,
                                             bias=bq_sb[:, mt:mt + 1],
                                             scale=1.0 / WSC)

                def emit_sr(rlo, rhi):
                    # taps accumulate in a bf16 scratch; the last tap writes
                    # the fp8 j-major cT tile consumed by the k/v DR matmuls
                    for ct in range(2):
                        h4 = h3[:, ct, :].rearrange("p (h a w b) -> p h a w b",
                                                    a=2, b=2, h=28, w=28)
                        cw3 = cw[:].rearrange("p (h w) -> p h w", w=28)
                        c3 = c8v[:, ct, :].rearrange("p (h w) -> p h w", w=28)
                        nc.vector.tensor_scalar_mul(cw3[:, rlo:rhi, :],
                                                    h4[:, rlo:rhi, 0, :, 0],
                                                    srw_sb[ct][:, 0:1])
                        for ky, kx in ((0, 1), (1, 0), (1, 1)):
                            ti = ky * 2 + kx
                            dst = cw3 if ti != 3 else c3
                            with nc.allow_low_precision("fp8 SR output"):
                                nc.vector.scalar_tensor_tensor(
                                    dst[:, rlo:rhi, :], h4[:, rlo:rhi, ky, :, kx],
                                    srw_sb[ct][:, ti:ti + 1],
                                    cw3[:, rlo:rhi, :], op0=Alu.mult, op1=Alu.add)

                def emit_k(n0, nsz):
                    for mt in range(2):
                        ps = mmp.tile([128, NT], F32, tag="mm")
                        nc.tensor.matmul(ps[:, :nsz], wk3[:, :, mt * 128:(mt + 1) * 128],
                                         c8v[:, :, n0:n0 + nsz], start=True,
                                         stop=True, perf_mode=DR)
                        nc.vector.tensor_scalar(k_sb[mt][:, n0:n0 + nsz], ps[:, :nsz],
                                                1.0 / WSC, bk_sb[:, mt:mt + 1],
                                                op0=Alu.mult, op1=Alu.add)

                def emit_v(kts):
                    # v bias is folded into bo host-side (Wo @ bv is constant
                    # after softmax normalization), so the evict is one strided
                    # ACT copy into the ones-augmented head-packed layout
                    for kt in kts:
                        ps = mmp.tile([128, NT], F32, tag="mm")
                        nc.tensor.matmul(ps[0:KT, 0:C],
                                         c8v[:, :, kt * KT:(kt + 1) * KT],
                                         wv3[:, :, :], start=True, stop=True,
                                         perf_mode=DR)
                        vv = v_sb[0:KT, kt * 260:(kt + 1) * 260].rearrange(
                            "p (h x) -> p h x", h=NH)
                        nc.scalar.activation(vv[:, :, 0:64],
                                             ps[0:KT, 0:C].rearrange(
                                                 "p (h x) -> p h x", h=NH),
                                             Act.Identity, scale=1.0 / WSC)
                        nc.vector.memset(vv[:, :, 64:65], 1.0)

                for nt in range(4):
                    emit_q(nt)
                emit_sr(0, 16)
                emit_k(0, 448)
                emit_v(range(4))
                for nt in range(4, NNT):
                    emit_q(nt)
                emit_sr(16, 28)
                emit_k(448, 336)
                # fold q/k to [32, j=2, n] fp8 per head (gpsimd DMA casts and
                # moves partitions) so the score matmuls can run DoubleRow;
                # emitted before the second v batch so the casts overlap it
                for h in range(NH):
                    ht, hr = h // 2, (h % 2) * 64
                    for j in range(2):
                        nc.gpsimd.dma_start(
                            out=q8v[h][:, j, :],
                            in_=q_sb[ht][hr + 32 * j:hr + 32 * j + 32, :])
                        nc.gpsimd.dma_start(
                            out=k8v[h][:, j, :],
                            in_=k_sb[ht][hr + 32 * j:hr + 32 * j + 32, :])
                emit_v(range(4, NKT))

            for m in DW_PE_TILES:
                build_diag(f"m{m}", dw9_sb[m])

            # flash attention (heads interleaved for PE row-group packing;
            # rel-pos bias applied as exp(s)*exp(rp) with host-precomputed
            # exp(rp) multiplied in on DVE/Pool). o_cat is written raw per
            # head (Pool evict) with denominator rows batched per q-tile:
            # one DVE reciprocal over [NH, NT], PE broadcast into [128, NT]
            # PSUM per ct-tile, then one in-place DVE normalize per ct-tile.
            with ExitStack() as pctx:
                rpp = pctx.enter_context(tc.tile_pool(name="rp", bufs=3))
                ppp = pctx.enter_context(tc.tile_pool(name="pexp", bufs=3))
                sps = pctx.enter_context(tc.tile_pool(name="spsum", bufs=2, space="PSUM"))
                ops = pctx.enter_context(tc.tile_pool(name="opsum", bufs=2, space="PSUM"))
                rps = pctx.enter_context(tc.tile_pool(name="rpsum", bufs=1, space="PSUM"))
                rsp = pctx.enter_context(tc.tile_pool(name="rsb", bufs=2))
                # software-pipelined by one step: scores/exp/p-mul for item
                # i+1 are emitted before AV of item i, so the in-order PE
                # queue never parks behind an AV that waits on DVE, and the
                # exp stream (the phase bottleneck) stays fed.
                # persistent denominator-rows tile: heads of a pair write
                # partitions 64/96; rows 65..95 zeroed once so the broadcast
                # matmul's contraction over [64,97) sees no garbage
                rrq = rsp.tile([97, NT], BF, tag="rrq", name="rrq", bufs=1)
                nc.vector.memset(rrq[64:97, :], 0.0)
                ln2_st0 = lnsp.tile([NNT, NT], F32, tag="ln2st0", name="ln2_st0")

                def emit_scores(qt, h):
                    cs = slice(qt * NT, (qt + 1) * NT)
                    ht, hr = h // 2, (h % 2) * 64
                    rp_t = rpp.tile([KT, NKT, NT], BF, name="rp_t")
                    nc.sync.dma_start(out=rp_t[:], in_=rpT_d.ap()[h, qt])
                    p_t = ppp.tile([KT, NKT, NT], BF, name="p_t")
                    p_f = p_t[:].rearrange("p a b -> p (a b)")
                    r_f = rp_t[:].rearrange("p a b -> p (a b)")
                    for g0, glen in ((0, 2), (2, 2), (4, 2), (6, 1)):
                        s_ps = sps.tile([KT, 1024], F32, name="s_ps")
                        s3v = s_ps[:].rearrange("p (a b) -> p a b", b=512)
                        for j in range(glen):
                            kt = g0 + j
                            nc.tensor.matmul(
                                s_ps[:, j * 512:j * 512 + NT],
                                k8v[h][:, :, kt * KT:(kt + 1) * KT],
                                q8v[h][:, :, cs], start=True, stop=True,
                                perf_mode=DR)
                        gs = slice(g0 * NT, (g0 + glen) * NT)
                        if glen == 1:
                            # Schraudolph fast-exp on DVE for the lone tail
                            # group: exp(s) ~= bitcast_f32(int32(A*s + B)),
                            # |rel err| <= ~4%; shifts ~15us of exp off the
                            # ACT engine, which gates the attention phase.
                            eti = rsp.tile([KT, NT], I32, tag="eti", name="eti",
                                           bufs=3)
                            nc.vector.tensor_scalar(eti[:], s_ps[:, 0:NT],
                                                    FE_A, FE_B,
                                                    op0=Alu.mult, op1=Alu.add)
                            nc.gpsimd.tensor_mul(p_f[:, gs],
                                                 eti[:].bitcast(F32), r_f[:, gs])
                            continue
                        et = rsp.tile([KT, 2 * NT], BF, tag="et", name="et", bufs=3)
                        e3v = et[:].rearrange("p (a b) -> p a b", b=NT)
                        nc.scalar.activation(e3v[:, :glen, :], s3v[:, :glen, 0:NT],
                                             Act.Exp)
                        eng = nc.vector if g0 == 0 else nc.gpsimd
                        eng.tensor_mul(p_f[:, gs], et[:, :glen * NT], r_f[:, gs])
                    return p_t

                def emit_av(qt, h, p_t):
                    cs = slice(qt * NT, (qt + 1) * NT)
                    ht, hr = h // 2, (h % 2) * 64
                    o_ps = ops.tile([65, NT], F32, name="o_ps")
                    for kt in range(NKT):
                        nc.tensor.matmul(o_ps[:],
                                         v_sb[0:KT, kt * 260 + h * 65: kt * 260 + (h + 1) * 65],
                                         p_t[:, kt, :], start=(kt == 0), stop=(kt == NKT - 1))
                    # reciprocal of the denominator row straight out of PSUM
                    # (written at partition 64, a legal matmul-rhs base); raw
                    # head output evicted into o_cat and normalized in-place
                    # per ct-tile once both heads have landed
                    with nc.allow_low_precision("bf16 softmax denom row"):
                        nc.vector.reciprocal(rrq[64 + 32 * (h % 2):
                                                 65 + 32 * (h % 2), :],
                                             o_ps[64:65, :])
                    nc.vector.tensor_copy(o_cat[ht][hr:hr + 64, cs],
                                          o_ps[0:64, :])
                    if h % 2 == 1:
                        ct = h // 2
                        rb_ps = rps.tile([128, NT], F32, name="rb_ps")
                        nc.tensor.matmul(rb_ps[:], sel2[64:97, :], rrq[64:97, :])
                        nc.vector.tensor_mul(o_cat[ct][:, cs], o_cat[ct][:, cs],
                                             rb_ps[:])
                    if h != NH - 1:
                        return
                    # q-tile tail: wo projection (+residual) and LN2 squares
                    for mt in range(2):
                        ps = rps.tile([128, NT], F32, name="rb_ps")
                        for kt in range(2):
                            nc.tensor.matmul(ps[:], wo_sb[kt][:, mt * 128:(mt + 1) * 128],
                                             o_cat[kt][:, cs], start=(kt == 0), stop=(kt == 1))
                        nc.vector.scalar_tensor_tensor(xres[mt][:, cs], ps[:],
                                                       bo_sb[:, mt:mt + 1],
                                                       xres[mt][:, cs],
                                                       op0=Alu.add, op1=Alu.add)
                        nc.gpsimd.tensor_mul(sqbuf[mt][:, cs], xres[mt][:, cs],
                                             xres[mt][:, cs])
                        nc.gpsimd.tensor_copy(xstage[mt][:, cs], xres[mt][:, cs])
                        # LN2 mean stats interleaved: one accumulation group
                        # spanning the whole attention, finishing with qt=6
                        nc.tensor.matmul(ln2_st0[:],
                                         zsel_b[:, NNT - 1 - qt:2 * NNT - 1 - qt],
                                         xstage[mt][:, cs],
                                         start=(qt == 0 and mt == 0),
                                         stop=(qt == NNT - 1 and mt == 1))

                pending = None
                for qt in range(NNT):
                    for h in range(NH):
                        p_t = emit_scores(qt, h)
                        if pending is not None:
                            emit_av(*pending)
                        pending = (qt, h, p_t)
                emit_av(*pending)

            layer_norm("2" + it, sq_src=sqbuf, st0_src=ln2_st0)
            ctx.close()

        # ================= stage 2: LN2 + conv-MLP + blk dwconv =================
        # dwconv inputs are x-padded to width 58 (zero cols 0 and 57) so all
        # taps are full-width and matmul outputs stay flat 2D.
        WP = WS + 2

        def run_stage2(it):
            ctx = ExitStack()
            mpool = ctx.enter_context(tc.tile_pool(name="mlp_ps", bufs=3, space="PSUM"))
            dps = ctx.enter_context(tc.tile_pool(name="dw_ps", bufs=2, space="PSUM"))
            upool = ctx.enter_context(tc.tile_pool(name="u", bufs=5))
            accp = ctx.enter_context(tc.tile_pool(name="dwacc", bufs=2))
            y2p = ctx.enter_context(tc.tile_pool(name="y2", bufs=1))
            y28 = y2p.tile([128, 8 * N], F8, tag="y28", name="y28")
            y2q = y28[:].rearrange("p (m n) -> p m n", m=8)
            y2 = [y2q[:, m, :] for m in range(8)]
            x3p = [y2p.tile([128, HS * WP], F32, tag=f"x3p{t}", name=f"x3p{t}")
                   for t in range(2)]
            x3b = [y2p.tile([128, HS * WP], BF, tag=f"x3b{t}", name=f"x3b{t}")
                   for t in range(2)]


            def tap_windows(r0):
                wins = []
                for dy, dx in TAPS:
                    rlo = max(r0, 1 if dy < 0 else 0)
                    rhi = min(r0 + 8, HS - (1 if dy > 0 else 0))
                    if rlo < rhi:
                        wins.append((dy, dx, rlo, rhi))
                return wins

            def dw_pe(src3, key, bias_col, dst):
                """3x3 depthwise conv of padded bf16 src3 [128,56,58] via PE
                diag matmuls; gelu evict with bias -> dst fp8."""
                diag = diag_all[key]
                for nt in range(NNT):
                    ps = dps.tile([128, NT], F32, tag="dwps", name="dwps")
                    p3 = ps[:].rearrange("p (h w) -> p h w", w=WS)
                    r0 = nt * 8
                    nc.tensor.matmul(ps[:], diag[4][:], src3[:, r0:r0 + 8, 1:57],
                                     start=True, stop=False)
                    wins = tap_windows(r0)
                    for ti, (dy, dx, rlo, rhi) in enumerate(wins):
                        nc.tensor.matmul(
                            ps[:, (rlo - r0) * WS:(rhi - r0) * WS],
                            diag[tap_idx(dy, dx)][:],
                            src3[:, rlo + dy:rhi + dy, 1 + dx:57 + dx],
                            start=False, stop=(ti == len(wins) - 1))
                    with nc.allow_low_precision("fp8 y2"):
                        nc.scalar.activation(dst[:, r0 * WS:(r0 + 8) * WS], ps[:],
                                             Act.Gelu, bias=bias_col)

            def dw_pool(src3, w9_sb, bias_col, dst, r0=0, r1=HS):
                """3x3 depthwise conv rows [r0, r1) on Pool: GPSIMD has no
                per-partition-scalar ops, so each tap is mul-into-scratch +
                add-into-acc with the weight as a stride-0 broadcast AP."""
                acc = accp.tile([128, N], BF, name="acc")
                a3 = acc[:].rearrange("p (h w) -> p h w", w=WS)
                tmp = accp.tile([128, N], BF, name="ptmp", tag="ptmp", bufs=2)
                t3 = tmp[:].rearrange("p (h w) -> p h w", w=WS)
                with nc.allow_low_precision("bf16 dwconv accumulator"):
                    wb = w9_sb[:, 4:5].broadcast_to([128, r1 - r0, WS])
                    nc.gpsimd.tensor_mul(a3[:, r0:r1, :], src3[:, r0:r1, 1:57], wb)
                    for dy, dx in TAPS:
                        rlo = max(r0, 1 if dy < 0 else 0)
                        rhi = min(r1, HS - (1 if dy > 0 else 0))
                        t = tap_idx(dy, dx)
                        wb = w9_sb[:, t:t + 1].broadcast_to([128, rhi - rlo, WS])
                        nc.gpsimd.tensor_mul(
                            t3[:, rlo:rhi, :],
                            src3[:, rlo + dy:rhi + dy, 1 + dx:57 + dx], wb)
                        nc.gpsimd.tensor_add(a3[:, rlo:rhi, :], a3[:, rlo:rhi, :],
                                             t3[:, rlo:rhi, :])
                with nc.allow_low_precision("fp8 y2"):
                    nc.scalar.activation(dst[:, r0 * WS:r1 * WS],
                                         acc[:, r0 * WS:r1 * WS], Act.Gelu,
                                         bias=bias_col)

            def dw_dve(src3, w9_sb, bias_col, dst, eng, r0=0, r1=HS):
                """3x3 depthwise conv rows [r0, r1) on DVE or Pool: center-tap
                init (+bias), 8 stt taps. bf16 accumulator keeps DVE in the 2x
                16-bit mode."""
                acc = accp.tile([128, N], BF, name="acc")
                a3 = acc[:].rearrange("p (h w) -> p h w", w=WS)
                with nc.allow_low_precision("bf16 dwconv accumulator"):
                    eng.tensor_scalar(a3[:, r0:r1, :], src3[:, r0:r1, 1:57],
                                      w9_sb[:, 4:5],
                                      bias_col, op0=Alu.mult, op1=Alu.add)
                    for dy, dx in TAPS:
                        rlo = max(r0, 1 if dy < 0 else 0)
                        rhi = min(r1, HS - (1 if dy > 0 else 0))
                        t = tap_idx(dy, dx)
                        eng.scalar_tensor_tensor(
                            a3[:, rlo:rhi, :], src3[:, rlo + dy:rhi + dy, 1 + dx:57 + dx],
                            w9_sb[:, t:t + 1], a3[:, rlo:rhi, :], op0=Alu.mult, op1=Alu.add)
                with nc.allow_low_precision("fp8 y2"):
                    nc.scalar.activation(dst[:, r0 * WS:r1 * WS],
                                         acc[:, r0 * WS:r1 * WS], Act.Gelu)

            # emit all w1 projections first so the PE stream isn't blocked
            # behind per-tile dwconvs; slow-engine (Pool/DVE) dw tiles come
            # first so their long serial tap chains start as early as
            # possible, and each runs as two row-halves so w2 for the first
            # row blocks can start while second halves are still in flight
            u3s = {}
            pad1 = onespad[:].rearrange("p (h o) -> p h o", o=1)
            slow = [m for m in range(8) if m not in DW_PE_TILES]
            for m in slow + list(DW_PE_TILES):
                u = upool.tile([128, HS * WP], BF, name="u")
                u3 = u[:].rearrange("p (h w) -> p h w", w=WP)
                u3s[m] = u3
                nc.vector.tensor_scalar_mul(u3[:, :, 0:1], pad1, ncol_sb[:, m:m + 1])
                nc.vector.tensor_scalar_mul(u3[:, :, 57:58], pad1, ncol_sb[:, m:m + 1])
                for nt0 in range(0, NNT, 2):
                    npair = min(2, NNT - nt0)
                    ps = mpool.tile([128, 1024], F32, tag="mm", name="mmps")
                    for j in range(npair):
                        cs = slice((nt0 + j) * NT, (nt0 + j + 1) * NT)
                        nc.tensor.matmul(ps[:, j * 512:j * 512 + NT],
                                         w13[:, :, m * 128:(m + 1) * 128],
                                         h3[:, :, cs], start=True, stop=True,
                                         perf_mode=DR)
                    psv = ps[:].rearrange("p (j n) -> p j n", j=2)[:, 0:npair, 0:NT]
                    nc.scalar.activation(
                        u3[:, nt0 * 8:(nt0 + npair) * 8, 1:57], psv,
                        Act.Gelu, bias=b1_sb[:, m:m + 1], scale=1.0 / WSC)
            RSPLIT = 32
            for half in ((0, RSPLIT), (RSPLIT, HS)):
                for m in slow:
                    if m in DW_POOL_TILES:
                        dw_pool(u3s[m], dw9_sb[m], dwb_sb[:, m:m + 1], y2[m],
                                r0=half[0], r1=half[1])
                    else:
                        dw_dve(u3s[m], dw9_sb[m], dwb_sb[:, m:m + 1], y2[m],
                               nc.vector, r0=half[0], r1=half[1])
            for m in DW_PE_TILES:
                dw_pe(u3s[m], f"m{m}", dwb_sb[:, m:m + 1], y2[m])

            # diag matrices for the PE blk tiles (DVE/ACT, cheap, deps ready)
            for ct in range(2):
                if ct not in BLK_SLOW_TILES:
                    build_diag(f"b{ct}", bk9_sb[ct])

            # final blk dwconv row-block: Pool tiles accumulate 8 taps from
            # the exact f32 xp3 (bf16 acc); PE tiles run diag matmuls over
            # xb3. Exact-fp32 center/residual fused in the evict:
            # f = acc + (1 + w_center) * x3.
            def emit_blk_nt(ct, nt):
                xp3 = x3p[ct][:].rearrange("p (h w) -> p h w", w=WP)
                r0 = nt * 8
                fo = accp.tile([128, NT], F32, tag="fout", name="fout", bufs=3)
                f3 = fo[:].rearrange("p (h w) -> p h w", w=WS)
                if ct in BLK_SLOW_TILES:
                    acc = accp.tile([128, NT], BF, tag="blkacc", name="blkacc",
                                    bufs=2)
                    a3 = acc[:].rearrange("p (h w) -> p h w", w=WS)
                    with nc.allow_low_precision("bf16 blk dwconv accumulator"):
                        nc.vector.tensor_scalar(
                            a3[:, :, :], xp3[:, r0:r0 + 8, 0:56],
                            bk9_sb[ct][:, 3:4], bkb_col[:, ct:ct + 1],
                            op0=Alu.mult, op1=Alu.add)
                        for dy, dx in TAPS:
                            if (dy, dx) == (0, -1):
                                continue
                            rlo = max(r0, 1 if dy < 0 else 0)
                            rhi = min(r0 + 8, HS - (1 if dy > 0 else 0))
                            t = tap_idx(dy, dx)
                            nc.vector.scalar_tensor_tensor(
                                a3[:, rlo - r0:rhi - r0, :],
                                xp3[:, rlo + dy:rhi + dy, 1 + dx:57 + dx],
                                bk9_sb[ct][:, t:t + 1],
                                a3[:, rlo - r0:rhi - r0, :],
                                op0=Alu.mult, op1=Alu.add)
                    nc.vector.scalar_tensor_tensor(
                        f3[:, :, :], xp3[:, r0:r0 + 8, 1:57], bk9_sb[ct][:, 4:5],
                        a3[:, :, :], op0=Alu.mult, op1=Alu.add)
                else:
                    diag = diag_all[f"b{ct}"]
                    xb3 = x3b[ct][:].rearrange("p (h w) -> p h w", w=WP)
                    ps = dps.tile([128, NT], F32, tag="dwps", name="blkps")
                    p3 = ps[:].rearrange("p (h w) -> p h w", w=WS)
                    nc.tensor.matmul(ps[:], bkb_row[0:1, ct * 128:(ct + 1) * 128],
                                     onesn[:], start=True, stop=False)
                    wins = tap_windows(r0)
                    for ti, (dy, dx, rlo, rhi) in enumerate(wins):
                        dg = diag[tap_idx(dy, dx)]
                        nc.tensor.matmul(
                            p3[:, rlo - r0:rhi - r0, :], dg[:],
                            xb3[:, rlo + dy:rhi + dy, 1 + dx:57 + dx],
                            start=False, stop=(ti == len(wins) - 1))
                    nc.vector.scalar_tensor_tensor(
                        f3[:, :, :], xp3[:, r0:r0 + 8, 1:57], bk9_sb[ct][:, 4:5],
                        ps[:].rearrange("p (h w) -> p h w", w=WS),
                        op0=Alu.mult, op1=Alu.add)
                nc.sync.dma_start(
                    out=fT_d[ct * 128:(ct + 1) * 128, r0 * WS:(r0 + 8) * WS],
                    in_=fo[:])

            # w2 (+bn2/pbn folded bias) + residual -> x3p (padded, f32), bf16
            # copy only for PE blk tiles; blkdw for row-block nt-1 is emitted
            # right after w2's nt lands so the final conv pipelines with w2
            # instead of forming a serial tail.
            for mt in range(2):
                xp3 = x3p[mt][:].rearrange("p (h w) -> p h w", w=WP)
                nc.vector.memset(xp3[:, :, 0:1], 0.0)
                nc.vector.memset(xp3[:, :, 57:58], 0.0)
                if mt not in BLK_SLOW_TILES:
                    xb3 = x3b[mt][:].rearrange("p (h w) -> p h w", w=WP)
                    nc.vector.memset(xb3[:, :, 0:1], 0.0)
                    nc.vector.memset(xb3[:, :, 57:58], 0.0)
            for nt in range(NNT):
                for mt in range(2):
                    xp3 = x3p[mt][:].rearrange("p (h w) -> p h w", w=WP)
                    cs = slice(nt * NT, (nt + 1) * NT)
                    ps = mpool.tile([128, NT], F32, tag="mm", name="mmps2")
                    # bias rides the accumulation as a K=1 ones matmul (the
                    # 1/32 weight descale in the evict leaves no slot for it)
                    nc.tensor.matmul(ps[:], b2rr[0:1, mt * 128:(mt + 1) * 128],
                                     onesn[:], start=True, stop=False,
                                     skip_group_check=True)
                    for q in range(4):
                        nc.tensor.matmul(ps[:], w23[:, q, :, mt * 128:(mt + 1) * 128],
                                         y2q[:, 2 * q:2 * q + 2, cs], start=False,
                                         stop=(q == 3), perf_mode=DR,
                                         skip_group_check=True)
                    nc.vector.scalar_tensor_tensor(
                        xp3[:, nt * 8:(nt + 1) * 8, 1:57], ps[:], 1.0 / WSC,
                        xres[mt][:, cs], op0=Alu.mult, op1=Alu.add)
                    if mt not in BLK_SLOW_TILES:
                        xb3 = x3b[mt][:].rearrange("p (h w) -> p h w", w=WP)
                        nc.gpsimd.tensor_copy(xb3[:, nt * 8:(nt + 1) * 8, 1:57],
                                              xp3[:, nt * 8:(nt + 1) * 8, 1:57])
                if nt >= 1:
                    for ct in range(2):
                        emit_blk_nt(ct, nt - 1)
            for ct in range(2):
                emit_blk_nt(ct, NNT - 1)
            ctx.close()

        for it in range(iters):
            body(f"_i{it}")

    nc.compile()
    return nc


_CACHE = {}


def _get_program():
    if "nc" not in _CACHE:
        _CACHE["nc"] = _build_program()
    return _CACHE["nc"]


def _prep_inputs(inputs):
    f64 = np.float64
    g1 = inputs["ln1_g"].astype(f64); b1ln = inputs["ln1_b"].astype(f64)
    g2 = inputs["ln2_g"].astype(f64); b2ln = inputs["ln2_b"].astype(f64)
    scale = DH ** -0.25

    def bn_ac(g, b, m, v):
        a = np.asarray(g, f64) / np.sqrt(np.asarray(v, f64) + EPS)
        return a, np.asarray(b, f64) - np.asarray(m, f64) * a

    wq = np.asarray(inputs["wq"], f64); wk = np.asarray(inputs["wk"], f64)
    wv = np.asarray(inputs["wv"], f64); wo = np.asarray(inputs["wo"], f64)

    wq_eff = wq * g1[None, :] * scale
    bq_eff = (wq @ b1ln + np.asarray(inputs["bq"], f64)) * scale

    sa, sc = bn_ac(inputs["srbn_g"], inputs["srbn_b"], inputs["srbn_m"], inputs["srbn_v"])
    srw4 = np.asarray(inputs["sr_w"], f64).reshape(C, 4)  # [c, ky*2+kx]
    srw_eff = srw4 * (g1 * sa)[:, None]
    d_const = sa * (b1ln * srw4.sum(1) + np.asarray(inputs["sr_b"], f64)) + sc
    bk_eff = (wk @ d_const + np.asarray(inputs["bk"], f64)) * scale
    bv_eff = wv @ d_const + np.asarray(inputs["bv"], f64)
    bo_eff = np.asarray(inputs["bo"], f64) + wo @ bv_eff

    w1 = np.asarray(inputs["w1"], f64)
    w1_eff = w1 * g2[None, :]
    b1_eff = w1 @ b2ln + np.asarray(inputs["b1"], f64)
    a1_, c1_ = bn_ac(inputs["bn1_g"], inputs["bn1_b"], inputs["bn1_m"], inputs["bn1_v"])

    dw9 = np.asarray(inputs["dw_w"], f64).reshape(HID, 9).copy()
    dw9[:, 4] += 1.0  # residual fold
    # bn1 fold: dw(a1*u + c1) + dwb = (dw9*a1)(u) + (dwb + c1*sum(dw9)) with
    # u padded by -c1/a1 so the conv's zero padding stays exact at borders
    dwb = np.asarray(inputs["dw_b"], f64) + c1_ * dw9.sum(1)
    ncol = -c1_ / a1_
    dw9 = dw9 * a1_[:, None]

    pa, pc = bn_ac(inputs["pbn_g"], inputs["pbn_b"], inputs["pbn_m"], inputs["pbn_v"])
    a2_, c2_ = bn_ac(inputs["bn2_g"], inputs["bn2_b"], inputs["bn2_m"], inputs["bn2_v"])
    w2 = np.asarray(inputs["w2"], f64)
    w2_eff = (w2 * pa[None, :]) * a2_[:, None]
    b2_eff = a2_ * (w2 @ pc + np.asarray(inputs["b2"], f64)) + c2_

    bk9 = np.asarray(inputs["blkdw_w"], f64).reshape(C, 9).copy()
    bk9[:, 4] += 1.0
    bkb = np.asarray(inputs["blkdw_b"], f64)

    bf = lambda a: np.ascontiguousarray(np.asarray(a, np.float32)).astype(BF16)
    f32 = lambda a: np.ascontiguousarray(np.asarray(a, np.float32))
    WSC = 32.0
    F8NP = ml_dtypes.float8_e4m3

    def f8w2(a):  # [256, M] -> j-major [128, 2, M] fp8, pre-scaled x32
        a = np.asarray(a, f64) * WSC
        a = np.clip(a, -224.0, 224.0).astype(np.float32)
        return np.ascontiguousarray(
            a.reshape(2, 128, -1).transpose(1, 0, 2).reshape(128, -1)).astype(F8NP)

    def f8w8(a):  # [1024, M] -> [128, q=4, j=2, M] fp8, pre-scaled x32
        a = np.asarray(a, f64) * WSC
        a = np.clip(a, -224.0, 224.0).astype(np.float32)
        M = a.shape[1]
        return np.ascontiguousarray(
            a.reshape(4, 2, 128, M).transpose(2, 0, 1, 3).reshape(128, -1)).astype(F8NP)

    shared = {
        "rpT": np.ascontiguousarray(
            np.exp(np.asarray(inputs["relative_pos"], np.float64))
            .reshape(NH, NNT, NT, NKT, KT).transpose(0, 1, 4, 3, 2)).astype(BF16),
        "wq8": f8w2(wq_eff.T), "wk8": f8w2(wk.T * scale), "wv8": f8w2(wv.T),
        "woT": bf(wo.T),
        "w18": f8w2(w1_eff.T), "w28": f8w8(w2_eff.T),
        "b2rr": bf(WSC * b2_eff[None, :]),
        "bq": f32(bq_eff), "bk": f32(bk_eff),
        "bo": f32(bo_eff), "b1": f32(b1_eff), "ncol": f32(ncol),
        "srw": f32(srw_eff), "dw9": f32(dw9), "dwb": f32(dwb),
        "bk9": f32(bk9), "bkb": bf(bkb[None, :]), "bkbc": f32(bkb),
        "eyeb": np.eye(128, dtype=np.float32).astype(BF16),
        "sselw": np.repeat(np.eye(NNT, dtype=np.float32), 128, axis=1).astype(BF16),
    }
    x = np.asarray(inputs["x"], np.float32)
    in_maps = []
    for b in range(B):
        m = dict(shared)
        m["xT"] = np.ascontiguousarray(x[b].T)
        in_maps.append(m)
    return in_maps


def kernel(**inputs):
    from concourse.bass_utils import run_bass_kernel_spmd
    nc = _get_program()
    in_maps = _prep_inputs(inputs)
    res = run_bass_kernel_spmd(nc, in_maps, core_ids=list(range(B)))
    out = np.stack([res.results[b]["fT"].T for b in range(B)], axis=0)
    return np.ascontiguousarray(out, dtype=np.float32)

